# revision 1
# baseline (speedup 1.0000x reference)
"""CapsuleLayer dynamic-routing kernel for 8 Trainium2 NeuronCores.

I-sharding: each core owns 144 of the 1152 input capsules.
  - inputs_hat computed per-i on PE: out[b=128,(n,d)=512] = X_i[8,128].T @ W_i[8,512],
    with a parallel PSUM-accumulation chain building s0_partial = sum_i hat_i.
  - hat kept bf16 in SBUF [128(b), 144(i), 512(nd)]; never touches HBM.
  - Routing: batch on partitions -> softmax over n and reductions over d/i are
    free-dim DVE/ACT ops.
  - Cross-core: AllReduce of s_partial [128,512] fp32, 3x.
Every core computes the identical final output; core 0's is returned.
"""

import os
import numpy as np

import concourse.bass as bass
import concourse.bacc as bacc
import concourse.tile as tile
import concourse.mybir as mybir
from concourse import bass_utils

B, I, DIN = 128, 1152, 8
N, D = 32, 16
ND = N * D  # 512
NCORES = 8
IL = I // NCORES  # 144
EPS = 1e-7
ROUTINGS = 3
F32 = mybir.dt.float32
BF16 = mybir.dt.bfloat16
CH = 4    # i-chunk for X/W streaming in create
BI = 8    # i-block for routing passes


def _ap(ap: bass.AP, dims) -> bass.AP:
    """Rebuild `ap` with explicit free [step,count] dims (partition dim kept)."""
    return bass.AP(tensor=ap.tensor, offset=ap.offset, ap=[ap.ap[0]] + list(dims))


def build_nc():
    nc = bacc.Bacc(
        "TRN2",
        target_bir_lowering=False,
        debug=False,
        enable_asserts=True,
        num_devices=NCORES,
    )
    x_d = nc.dram_tensor("x", [DIN, IL, B], F32, kind="ExternalInput").ap()
    w_d = nc.dram_tensor("w", [DIN, IL, ND], F32, kind="ExternalInput").ap()
    out_d = nc.dram_tensor("out", [B, ND], F32, kind="ExternalOutput").ap()

    with tile.TileContext(nc) as tc:
        with (
            tc.tile_pool(name="big", bufs=1) as big,
            tc.tile_pool(name="stream", bufs=1) as stream,
            tc.tile_pool(name="work", bufs=1) as work,
            tc.tile_pool(name="ps", bufs=5, space="PSUM") as pspool,
            tc.tile_pool(name="ps0", bufs=1, space="PSUM") as ps0pool,
            tc.tile_pool(name="dram", bufs=1, space="DRAM") as dram,
        ):
            hat = big.tile([B, IL, ND], BF16)        # 147.5 KB/part
            bb = big.tile([B, IL, N], BF16)          # 9.2 KB
            ee = big.tile([B, IL, N], BF16)          # 9.2 KB
            big4 = big.tile([B, 4, ND], F32)         # 8.2 KB
            s_sb, outv, s_acc, tsq = (big4[:, j, :] for j in range(4))
            outbf = big.tile([B, ND], BF16)          # 1->4 KB
            smalls = big.tile([B, 8, N], F32)        # 4 KB
            s2, a1, r1, rt = (smalls[:, j, :] for j in range(4))
            eps_t = smalls[:, 4, 0:1]
            ssum = big.tile([B, IL], F32)            # ->4 KB

            nc.vector.memset(eps_t, EPS)
            nc.vector.memset(bb[:], 0.0)

            # ---------- create hat + s0 ----------
            s0ps = ps0pool.tile([B, ND], F32)
            for ic in range(IL // CH):
                wt = stream.tile([DIN, CH, ND], F32)
                xch = stream.tile([DIN, CH, B], F32, tag="xch")
                nc.sync.dma_start(out=wt[:], in_=w_d[:, ic * CH:(ic + 1) * CH, :])
                nc.sync.dma_start(out=xch[:], in_=x_d[:, ic * CH:(ic + 1) * CH, :])
                for j in range(CH):
                    i = ic * CH + j
                    ps = pspool.tile([B, ND], F32)
                    nc.tensor.matmul(
                        ps[:], lhsT=xch[:, j, :], rhs=wt[:, j, :],
                        start=True, stop=True,
                    )
                    nc.tensor.matmul(
                        s0ps[:], lhsT=xch[:, j, :], rhs=wt[:, j, :],
                        start=(i == 0), stop=(i == IL - 1),
                    )
                    if i % 2 == 0:
                        nc.scalar.copy(out=hat[:, i, :], in_=ps[:])
                    else:
                        nc.vector.tensor_copy(hat[:, i, :], ps[:])

            nc.scalar.copy(out=s_acc, in_=s0ps[:])
            nc.scalar.mul(out=s_acc, in_=s_acc, mul=1.0 / N)

            def allreduce_s():
                ar_in = dram.tile([B, ND], F32, tag="arin")
                ar_out = dram.tile([B, ND], F32, tag="arout")
                nc.gpsimd.dma_start(out=ar_in[:], in_=s_acc)
                nc.gpsimd.collective_compute(
                    "AllReduce",
                    mybir.AluOpType.add,
                    replica_groups=[list(range(NCORES))],
                    ins=[ar_in.opt()],
                    outs=[ar_out.opt()],
                )
                nc.gpsimd.dma_start(out=s_sb, in_=ar_out[:])

            def squash(last: bool):
                nc.vector.tensor_mul(tsq, s_sb, s_sb)
                nc.vector.reduce_sum(
                    out=s2, in_=_ap(tsq, [[D, N], [1, D]]),
                    axis=mybir.AxisListType.X, )
                nc.scalar.add(out=a1, in_=s2, add=1.0)
                nc.vector.reciprocal(out=r1, in_=a1)
                nc.vector.tensor_mul(r1, r1, s2)          # s2/(1+s2)
                nc.scalar.activation(
                    out=rt, in_=s2,
                    func=mybir.ActivationFunctionType.Sqrt,
                    bias=eps_t, scale=1.0, )
                nc.vector.reciprocal(out=rt, in_=rt)
                nc.vector.tensor_mul(r1, r1, rt)          # full scale [B,N]
                nc.vector.tensor_mul(
                    _ap(outv, [[D, N], [1, D]]),
                    _ap(s_sb, [[D, N], [1, D]]),
                    _ap(r1, [[1, N], [0, D]]), )
                if not last:
                    nc.vector.tensor_copy(outbf[:], outv)

            NBLK = IL // BI
            allreduce_s()
            for r in range(ROUTINGS):
                squash(last=(r == ROUTINGS - 1))
                if r == ROUTINGS - 1:
                    break
                # ---- bb += sum_d hat*out ----
                for blk in range(NBLK):
                    i0 = blk * BI
                    tmp = work.tile([B, BI, N, D], BF16, tag="tmp")
                    nc.vector.tensor_mul(
                        tmp[:],
                        _ap(hat[:, i0:i0 + BI, :], [[ND, BI], [D, N], [1, D]]),
                        _ap(outbf[:], [[0, BI], [D, N], [1, D]]), )
                    dl = work.tile([B, BI * N], F32, tag="dl")
                    nc.vector.reduce_sum(
                        out=dl[:], in_=tmp[:], axis=mybir.AxisListType.X)
                    bbs = _ap(bb[:, i0:i0 + BI, :], [[1, BI * N]])
                    nc.vector.tensor_add(bbs, bbs, dl[:])
                # ---- softmax over n ----
                nc.scalar.activation(
                    out=ee[:], in_=bb[:],
                    func=mybir.ActivationFunctionType.Exp,
                    bias=eps_t, scale=1.0, )
                nc.vector.reduce_sum(
                    out=ssum[:], in_=ee[:], axis=mybir.AxisListType.X)
                nc.vector.reciprocal(out=ssum[:], in_=ssum[:])
                nc.vector.tensor_mul(
                    ee[:], ee[:], _ap(ssum[:], [[1, IL], [0, N]]))
                # ---- s_acc = sum_i c*hat ----
                nc.vector.memset(s_acc, 0.0)
                for blk in range(NBLK):
                    i0 = blk * BI
                    tmp = work.tile([B, BI, N, D], BF16, tag="tmp")
                    # expand c over d on ScalarE (runs parallel to DVE) so the
                    # DVE multiply gets contiguous bf16 operands (2x mode)
                    cexp = work.tile([B, BI, N, D], BF16, tag="cexp")
                    nc.scalar.copy(
                        out=cexp[:],
                        in_=_ap(ee[:, i0:i0 + BI, :], [[N, BI], [1, N], [0, D]]), )
                    nc.vector.tensor_mul(
                        tmp[:],
                        _ap(hat[:, i0:i0 + BI, :], [[ND, BI], [D, N], [1, D]]),
                        cexp[:], )
                    # tsq slot doubles as the per-block s scratch (squash is
                    # not active during this pass)
                    nc.vector.reduce_sum(
                        out=tsq, in_=_ap(tmp[:], [[1, ND], [ND, BI]]),
                        axis=mybir.AxisListType.X, )
                    nc.vector.tensor_add(s_acc, s_acc, tsq)
                allreduce_s()

            nc.sync.dma_start(out=out_d[:], in_=outv)

    nc.compile()
    return nc


_NC_CACHE = None


def kernel(inputs: np.ndarray, W: np.ndarray) -> np.ndarray:
    global _NC_CACHE
    if _NC_CACHE is None:
        _NC_CACHE = build_nc()
    nc = _NC_CACHE

    inputs = np.ascontiguousarray(inputs, dtype=np.float32)
    W = np.ascontiguousarray(W, dtype=np.float32)
    in_maps = []
    for c in range(NCORES):
        sl = slice(c * IL, (c + 1) * IL)
        x_c = np.ascontiguousarray(inputs[:, sl, :].transpose(2, 1, 0))
        w_c = np.ascontiguousarray(
            W[:, sl, :, :].transpose(3, 1, 0, 2).reshape(DIN, IL, ND))
        in_maps.append({"x": x_c, "w": w_c})

    trace = bool(int(os.environ.get("CAPS_TRACE", "0")))
    res = bass_utils.run_bass_kernel_spmd(
        nc, in_maps, core_ids=list(range(NCORES)), trace=trace)
    if trace and res.exec_time_ns is not None:
        print(f"HW exec time: {res.exec_time_ns} ns")
    return res.results[0]["out"].reshape(B, N, D).astype(np.float32)



# revision 4
# speedup vs baseline: 1.9581x; 1.9581x over previous
"""CapsuleLayer dynamic-routing kernel for 8 Trainium2 NeuronCores.

I-sharding: each core owns 144 of the 1152 input capsules.

v2 design (vs v1 baseline at 1.41 ms):
  - All matmuls bf16 (fp32 LOW_HIGH mode was 4x slower per row).
  - hat stored [b, i, d, n] (d-major, n innermost) in bf16. Every routing
    multiply then has an innermost-contiguous 16-bit access pattern ->
    DVE 2x mode; and both reductions (over d and over i) become pure
    contiguous-halving in-place add trees (2x) instead of TensorReduce (1x).
  - s0 = (1/N) sum_i hat computed by a dedicated K=128 GEMM over (i,k)
    so its AllReduce overlaps the whole hat-creation phase.
  - Routing block-parallel across engines: DVE owns 7 blocks of 16 i,
    GpSimd owns 4 blocks of 8 i; ACT does exp + a share of PSUM drains.
  - bb logits and s accumulators in bf16 (gate is 2e-2).
Cross-core: AllReduce of s [128,512] bf16, 3x (first hidden under create).
Every core computes the identical final output; core 0's is returned.
"""

import os
import numpy as np
import ml_dtypes

import concourse.bass as bass
import concourse.bacc as bacc
import concourse.tile as tile
import concourse.mybir as mybir
from concourse import bass_utils

B, I, DIN = 128, 1152, 8
N, D = 32, 16
ND = N * D  # 512, flattened (d, n): nd = d*32 + n
NCORES = 8
IL = I // NCORES  # 144
EPS = 1e-7
F32 = mybir.dt.float32
BF16 = mybir.dt.bfloat16
CH = 2            # i-chunk for X/W streaming in create

# routing block split: DVE gets 7 blocks of 16 i, GpSimd 4 blocks of 8 i
DVE_BI, DVE_NBLK = 16, 7     # i 0..112
GP_BI, GP_NBLK = 8, 4        # i 112..144
GP_I0 = DVE_BI * DVE_NBLK


def _ap(ap: bass.AP, dims) -> bass.AP:
    """Rebuild `ap` with explicit free [step,count] dims (partition dim kept)."""
    return bass.AP(tensor=ap.tensor, offset=ap.offset, ap=[ap.ap[0]] + list(dims))


def build_nc():
    nc = bacc.Bacc(
        "TRN2",
        target_bir_lowering=False,
        debug=False,
        enable_asserts=True,
        num_devices=NCORES,
    )
    x_d = nc.dram_tensor("x", [DIN, IL, B], BF16, kind="ExternalInput").ap()
    w_d = nc.dram_tensor("w", [DIN, IL, ND], BF16, kind="ExternalInput").ap()
    x2_d = nc.dram_tensor("x2", [IL * DIN, B], BF16, kind="ExternalInput").ap()
    w2_d = nc.dram_tensor("w2", [IL * DIN, ND], BF16, kind="ExternalInput").ap()
    out_d = nc.dram_tensor("out", [B, ND], F32, kind="ExternalOutput").ap()

    AX = mybir.AxisListType.X
    ACT_COPY = mybir.ActivationFunctionType.Copy
    ACT_EXP = mybir.ActivationFunctionType.Exp
    ACT_SQRT = mybir.ActivationFunctionType.Sqrt

    with tile.TileContext(nc) as tc:
        with (
            tc.tile_pool(name="big", bufs=1) as big,
            tc.tile_pool(name="stream", bufs=2) as stream,
            tc.tile_pool(name="workbig", bufs=1) as workbig,
            tc.tile_pool(name="worksm", bufs=2) as worksm,
            tc.tile_pool(name="ps", bufs=5, space="PSUM") as pspool,
            tc.tile_pool(name="ps0", bufs=1, space="PSUM") as ps0pool,
            tc.tile_pool(name="dram", bufs=1, space="DRAM") as dram,
        ):
            hat = big.tile([B, IL, ND], BF16)           # 147.5 KB/part
            bb_dve = big.tile([B, GP_I0, N], BF16)      # 7 KB
            bb_gp = big.tile([B, IL - GP_I0, N], BF16)  # 2 KB
            tsq = big.tile([B, ND], F32)                # 2 KB
            outT = big.tile([B, ND], F32)               # 2 KB
            outbf_t = big.tile([B, 2, ND], BF16)        # 2 KB
            sacc = big.tile([B, 2, ND], BF16)           # 2 KB per-engine s acc
            sbf = big.tile([B, ND], BF16)               # 1 KB AR payload
            s_ar = big.tile([B, ND], BF16)              # 1 KB AR result
            smalls = big.tile([B, 5, N], F32)           # 0.6 KB
            s2, a1, r1, rt = (smalls[:, j, :] for j in range(4))
            eps_t = smalls[:, 4, 0:1]
            # persistent per-engine work tiles (single-buffered: same-engine
            # program order already serializes reuse)
            dprod = workbig.tile([B, DVE_BI, ND], BF16)  # 16 KB
            gprod = workbig.tile([B, GP_BI, ND], BF16)   # 8 KB

            nc.vector.memset(eps_t, EPS)

            # ---------- s0 GEMM:  s0[b, nd] = sum_(i,k) x2 * w2  ----------
            s0ps = ps0pool.tile([B, ND], F32)
            NK = IL * DIN // B  # 9 chunks of K=128
            for j in range(NK):
                x2c = stream.tile([B, B], BF16, tag="x2")
                w2c = stream.tile([B, ND], BF16, tag="w2")
                nc.sync.dma_start(out=x2c[:], in_=x2_d[j * B:(j + 1) * B, :])
                nc.sync.dma_start(out=w2c[:], in_=w2_d[j * B:(j + 1) * B, :])
                nc.tensor.matmul(
                    s0ps[:], lhsT=x2c[:], rhs=w2c[:],
                    start=(j == 0), stop=(j == NK - 1),
                )
            # s0 * (1/N) -> bf16 AR payload
            nc.scalar.activation(out=sbf[:], in_=s0ps[:], func=ACT_COPY,
                                 scale=1.0 / N)

            def allreduce_s():
                ar_in = dram.tile([B, ND], BF16, tag="arin")
                ar_out = dram.tile([B, ND], BF16, tag="arout")
                nc.gpsimd.dma_start(out=ar_in[:], in_=sbf[:])
                nc.gpsimd.collective_compute(
                    "AllReduce",
                    mybir.AluOpType.add,
                    replica_groups=[list(range(NCORES))],
                    ins=[ar_in.opt()],
                    outs=[ar_out.opt()],
                )
                nc.gpsimd.dma_start(out=s_ar[:], in_=ar_out[:])

            allreduce_s()  # AR1 (s0) overlaps the create loop below

            # ---------- create hat ----------
            for ic in range(IL // CH):
                wt = stream.tile([DIN, CH, ND], BF16, tag="wch")
                xch = stream.tile([DIN, CH, B], BF16, tag="xch")
                nc.sync.dma_start(out=wt[:], in_=w_d[:, ic * CH:(ic + 1) * CH, :])
                nc.sync.dma_start(out=xch[:], in_=x_d[:, ic * CH:(ic + 1) * CH, :])
                for j in range(CH):
                    i = ic * CH + j
                    ps = pspool.tile([B, ND], F32)
                    nc.tensor.matmul(
                        ps[:], lhsT=xch[:, j, :], rhs=wt[:, j, :],
                        start=True, stop=True,
                    )
                    if i % 2 == 0:
                        nc.vector.tensor_copy(hat[:, i, :], ps[:])
                    else:
                        nc.scalar.copy(out=hat[:, i, :], in_=ps[:])

            # ---------- squash: out from s (bf16) ----------
            def squash(r):
                last = (r == 2)
                s_in = s_ar[:]
                nc.vector.tensor_mul(tsq[:], s_in, s_in)     # f32 = bf16^2
                # s2[b,n] = sum_d tsq   (d: stride 32, count 16, innermost)
                nc.vector.reduce_sum(
                    out=s2, in_=_ap(tsq[:], [[1, N], [N, D]]), axis=AX)
                nc.scalar.add(out=a1, in_=s2, add=1.0)
                nc.vector.reciprocal(out=r1, in_=a1)
                nc.vector.tensor_mul(r1, r1, s2)              # s2/(1+s2)
                nc.scalar.activation(out=rt, in_=s2, func=ACT_SQRT,
                                     bias=eps_t, scale=1.0)
                nc.vector.reciprocal(out=rt, in_=rt)
                nc.vector.tensor_mul(r1, r1, rt)              # scale [B,N]
                if last:
                    # outT[b, n, d] = s[b, d, n] * r1[b, n]
                    nc.vector.tensor_mul(
                        _ap(outT[:], [[D, N], [1, D]]),
                        _ap(s_in, [[1, N], [N, D]]),
                        _ap(r1, [[1, N], [0, D]]))
                    return None
                ob = outbf_t[:, r % 2, :]
                nc.vector.tensor_mul(
                    ob, s_in, _ap(r1, [[0, D], [1, N]]))      # bcast over d
                return ob

            # ---------- routing ----------
            def route_block(eng, prod, bbs, ee_tag, rr_tag, cee_tag,
                            BIb, hat_blk, ob, it, first_blk, sslot):
                # -- b-pass: prod = hat * out (bcast over i) --
                eng.tensor_mul(prod[:], hat_blk, _ap(ob, [[0, BIb], [1, ND]]))
                # in-place d-tree: contiguous halving over nd slabs
                for half in (256, 128, 64, 32):
                    eng.tensor_add(prod[:, :, 0:half], prod[:, :, 0:half],
                                   prod[:, :, half:2 * half])
                dsum = prod[:, :, 0:N]
                if it == 0:
                    eng.tensor_copy(bbs, dsum)
                else:
                    eng.tensor_add(bbs, bbs, dsum)
                # -- softmax over n (local per (b,i)) --
                ee = worksm.tile([B, BIb, N], BF16, tag=ee_tag)
                rr = worksm.tile([B, BIb], F32, tag=rr_tag)
                nc.scalar.activation(out=ee[:], in_=bbs, func=ACT_EXP,
                                     bias=eps_t, scale=1.0)
                nc.vector.reduce_sum(out=rr[:], in_=ee[:], axis=AX)
                nc.vector.reciprocal(out=rr[:], in_=rr[:])
                cee = worksm.tile([B, BIb, N], BF16, tag=cee_tag)
                eng.tensor_mul(cee[:], ee[:], _ap(rr[:], [[1, BIb], [0, N]]))
                # -- s-pass: prod = hat * c (bcast over d) --
                eng.tensor_mul(prod[:], hat_blk,
                               _ap(cee[:], [[N, BIb], [0, D], [1, N]]))
                # in-place i-tree
                half = BIb // 2
                while half >= 1:
                    eng.tensor_add(prod[:, 0:half, :], prod[:, 0:half, :],
                                   prod[:, half:2 * half, :])
                    half //= 2
                isum = prod[:, 0, :]
                ss = sacc[:, sslot, :]
                if first_blk:
                    eng.tensor_copy(ss, isum)
                else:
                    eng.tensor_add(ss, ss, isum)

            for it in range(2):
                ob = squash(it)
                for blk in range(DVE_NBLK):
                    i0 = blk * DVE_BI
                    route_block(nc.vector, dprod, bb_dve[:, i0:i0 + DVE_BI, :],
                                "dee", "drr", "dcee", DVE_BI,
                                hat[:, i0:i0 + DVE_BI, :], ob, it, blk == 0, 0)
                for blk in range(GP_NBLK):
                    i0 = GP_I0 + blk * GP_BI
                    j0 = i0 - GP_I0
                    route_block(nc.gpsimd, gprod, bb_gp[:, j0:j0 + GP_BI, :],
                                "gee", "grr", "gcee", GP_BI,
                                hat[:, i0:i0 + GP_BI, :], ob, it, blk == 0, 1)
                # s = sacc_dve + sacc_gp (bf16) -> AR payload
                nc.vector.tensor_add(sbf[:], sacc[:, 0, :], sacc[:, 1, :])
                allreduce_s()

            squash(2)
            nc.sync.dma_start(out=out_d[:], in_=outT[:])

    nc.compile()
    return nc


_NC_CACHE = None


def kernel(inputs: np.ndarray, W: np.ndarray) -> np.ndarray:
    global _NC_CACHE
    if _NC_CACHE is None:
        _NC_CACHE = build_nc()
    nc = _NC_CACHE

    inputs = np.ascontiguousarray(inputs, dtype=np.float32)
    W = np.ascontiguousarray(W, dtype=np.float32)
    bf = ml_dtypes.bfloat16
    in_maps = []
    for c in range(NCORES):
        sl = slice(c * IL, (c + 1) * IL)
        xs = inputs[:, sl, :]                     # [B, IL, 8]
        ws = W[:, sl, :, :]                       # [N, IL, D, 8]
        x_c = np.ascontiguousarray(xs.transpose(2, 1, 0).astype(bf))
        w_c = np.ascontiguousarray(
            ws.transpose(3, 1, 2, 0).astype(bf)).reshape(DIN, IL, ND)
        x2_c = np.ascontiguousarray(
            xs.transpose(1, 2, 0).astype(bf)).reshape(IL * DIN, B)
        w2_c = np.ascontiguousarray(
            ws.transpose(1, 3, 2, 0).astype(bf)).reshape(IL * DIN, ND)
        in_maps.append({"x": x_c, "w": w_c, "x2": x2_c, "w2": w2_c})

    trace = bool(int(os.environ.get("CAPS_TRACE", "0")))
    res = bass_utils.run_bass_kernel_spmd(
        nc, in_maps, core_ids=list(range(NCORES)), trace=trace)
    if trace and res.exec_time_ns is not None:
        print(f"HW exec time: {res.exec_time_ns} ns")
    return res.results[0]["out"].reshape(B, N, D).astype(np.float32)


# revision 8
# speedup vs baseline: 2.4800x; 1.2665x over previous
"""CapsuleLayer dynamic-routing kernel for 8 Trainium2 NeuronCores.

I-sharding: each core owns 144 of the 1152 input capsules.

v2 design (vs v1 baseline at 1.41 ms):
  - All matmuls bf16 (fp32 LOW_HIGH mode was 4x slower per row).
  - hat stored [b, i, d, n] (d-major, n innermost) in bf16. Every routing
    multiply then has an innermost-contiguous 16-bit access pattern ->
    DVE 2x mode; and both reductions (over d and over i) become pure
    contiguous-halving in-place add trees (2x) instead of TensorReduce (1x).
  - s0 = (1/N) sum_i hat computed by a dedicated K=128 GEMM over (i,k)
    so its AllReduce overlaps the whole hat-creation phase.
  - Routing block-parallel across engines: DVE owns 7 blocks of 16 i,
    GpSimd owns 4 blocks of 8 i; ACT does exp + a share of PSUM drains.
  - bb logits and s accumulators in bf16 (gate is 2e-2).
Cross-core: AllReduce of s [128,512] bf16, 3x (first hidden under create).
Every core computes the identical final output; core 0's is returned.
"""

import os
import numpy as np
import ml_dtypes

import concourse.bass as bass
import concourse.bacc as bacc
import concourse.tile as tile
import concourse.mybir as mybir
from concourse import bass_utils

B, I, DIN = 128, 1152, 8
N, D = 32, 16
ND = N * D  # 512, flattened (d, n): nd = d*32 + n
NCORES = 8
IL = I // NCORES  # 144
EPS = 1e-7
F32 = mybir.dt.float32
BF16 = mybir.dt.bfloat16
CH = 2            # i-chunk for X/W streaming in create

# routing block split: DVE gets 7 blocks of 16 i, GpSimd 4 blocks of 8 i
DVE_BI, DVE_NBLK = 16, 9     # all i on DVE
GP_BI, GP_NBLK = 8, 0        # GP disabled (SBUF contention test)
GP_I0 = DVE_BI * DVE_NBLK


def _ap(ap: bass.AP, dims) -> bass.AP:
    """Rebuild `ap` with explicit free [step,count] dims (partition dim kept)."""
    return bass.AP(tensor=ap.tensor, offset=ap.offset, ap=[ap.ap[0]] + list(dims))


def build_nc():
    nc = bacc.Bacc(
        "TRN2",
        target_bir_lowering=False,
        debug=False,
        enable_asserts=True,
        num_devices=NCORES,
    )
    x_d = nc.dram_tensor("x", [DIN, IL, B], BF16, kind="ExternalInput").ap()
    w_d = nc.dram_tensor("w", [DIN, IL, ND], BF16, kind="ExternalInput").ap()
    x2_d = nc.dram_tensor("x2", [IL * DIN, B], BF16, kind="ExternalInput").ap()
    w2_d = nc.dram_tensor("w2", [IL * DIN, ND], BF16, kind="ExternalInput").ap()
    out_d = nc.dram_tensor("out", [B, ND], F32, kind="ExternalOutput").ap()

    AX = mybir.AxisListType.X
    ACT_COPY = mybir.ActivationFunctionType.Copy
    ACT_EXP = mybir.ActivationFunctionType.Exp
    ACT_SQRT = mybir.ActivationFunctionType.Sqrt

    with tile.TileContext(nc) as tc:
        with (
            tc.tile_pool(name="big", bufs=1) as big,
            tc.tile_pool(name="stream", bufs=2) as stream,
            tc.tile_pool(name="workbig", bufs=1) as workbig,
            tc.tile_pool(name="worksm", bufs=2) as worksm,
            tc.tile_pool(name="ps", bufs=5, space="PSUM") as pspool,
            tc.tile_pool(name="ps0", bufs=1, space="PSUM") as ps0pool,
            tc.tile_pool(name="dram", bufs=1, space="DRAM") as dram,
        ):
            hat = big.tile([B, IL, ND], BF16)           # 147.5 KB/part
            bb_dve = big.tile([B, GP_I0, N], BF16)      # 7 KB
            bb_gp = (big.tile([B, IL - GP_I0, N], BF16)
                     if GP_NBLK else None)              # 2 KB
            tsq = big.tile([B, ND], F32)                # 2 KB
            outT = big.tile([B, ND], F32)               # 2 KB
            outbf_t = big.tile([B, 2, ND], BF16)        # 2 KB
            sacc = big.tile([B, 2, ND], BF16)           # 2 KB per-engine s acc
            sbf = big.tile([B, ND], BF16)               # 1 KB AR payload
            s_ar = big.tile([B, ND], BF16)              # 1 KB AR result
            smalls = big.tile([B, 5, N], F32)           # 0.6 KB
            s2, a1, r1, rt = (smalls[:, j, :] for j in range(4))
            eps_t = smalls[:, 4, 0:1]
            # persistent per-engine work tiles (single-buffered: same-engine
            # program order already serializes reuse)
            dprod = workbig.tile([B, DVE_BI, ND], BF16)  # 16 KB
            gprod = (workbig.tile([B, GP_BI, ND], BF16)
                     if GP_NBLK else None)               # 8 KB

            nc.vector.memset(eps_t, EPS)

            # ---------- s0 GEMM:  s0[b, nd] = sum_(i,k) x2 * w2  ----------
            s0ps = ps0pool.tile([B, ND], F32)
            NK = IL * DIN // B  # 9 chunks of K=128
            for j in range(NK):
                x2c = stream.tile([B, B], BF16, tag="x2")
                w2c = stream.tile([B, ND], BF16, tag="w2")
                nc.sync.dma_start(out=x2c[:], in_=x2_d[j * B:(j + 1) * B, :])
                nc.sync.dma_start(out=w2c[:], in_=w2_d[j * B:(j + 1) * B, :])
                nc.tensor.matmul(
                    s0ps[:], lhsT=x2c[:], rhs=w2c[:],
                    start=(j == 0), stop=(j == NK - 1),
                )
            # s0 * (1/N) -> bf16 AR payload
            nc.scalar.activation(out=sbf[:], in_=s0ps[:], func=ACT_COPY,
                                 scale=1.0 / N)

            def allreduce_s():
                ar_in = dram.tile([B, ND], BF16, tag="arin")
                ar_out = dram.tile([B, ND], BF16, tag="arout")
                nc.gpsimd.dma_start(out=ar_in[:], in_=sbf[:])
                nc.gpsimd.collective_compute(
                    "AllReduce",
                    mybir.AluOpType.add,
                    replica_groups=[list(range(NCORES))],
                    ins=[ar_in.opt()],
                    outs=[ar_out.opt()],
                )
                nc.gpsimd.dma_start(out=s_ar[:], in_=ar_out[:])

            allreduce_s()  # AR1 (s0) overlaps the create loop below

            # ---------- create hat ----------
            for ic in range(IL // CH):
                wt = stream.tile([DIN, CH, ND], BF16, tag="wch")
                xch = stream.tile([DIN, CH, B], BF16, tag="xch")
                nc.sync.dma_start(out=wt[:], in_=w_d[:, ic * CH:(ic + 1) * CH, :])
                nc.sync.dma_start(out=xch[:], in_=x_d[:, ic * CH:(ic + 1) * CH, :])
                for j in range(CH):
                    i = ic * CH + j
                    ps = pspool.tile([B, ND], F32)
                    nc.tensor.matmul(
                        ps[:], lhsT=xch[:, j, :], rhs=wt[:, j, :],
                        start=True, stop=True,
                    )
                    if i % 2 == 0:
                        nc.vector.tensor_copy(hat[:, i, :], ps[:])
                    else:
                        nc.scalar.copy(out=hat[:, i, :], in_=ps[:])

            # ---------- squash: out from s (bf16) ----------
            def squash(r):
                last = (r == 2)
                s_in = s_ar[:]
                nc.vector.tensor_mul(tsq[:], s_in, s_in)     # f32 = bf16^2
                # s2[b,n] = sum_d tsq   (d: stride 32, count 16, innermost)
                nc.vector.reduce_sum(
                    out=s2, in_=_ap(tsq[:], [[1, N], [N, D]]), axis=AX)
                nc.scalar.add(out=a1, in_=s2, add=1.0)
                nc.vector.reciprocal(out=r1, in_=a1)
                nc.vector.tensor_mul(r1, r1, s2)              # s2/(1+s2)
                nc.scalar.activation(out=rt, in_=s2, func=ACT_SQRT,
                                     bias=eps_t, scale=1.0)
                nc.vector.reciprocal(out=rt, in_=rt)
                nc.vector.tensor_mul(r1, r1, rt)              # scale [B,N]
                if last:
                    # outT[b, n, d] = s[b, d, n] * r1[b, n]
                    nc.vector.tensor_mul(
                        _ap(outT[:], [[D, N], [1, D]]),
                        _ap(s_in, [[1, N], [N, D]]),
                        _ap(r1, [[1, N], [0, D]]))
                    return None
                ob = outbf_t[:, r % 2, :]
                nc.vector.tensor_mul(
                    ob, s_in, _ap(r1, [[0, D], [1, N]]))      # bcast over d
                return ob

            # ---------- routing ----------
            def route_block(eng, prod, bbs, ee_tag, rr_tag, cee_tag,
                            BIb, hat_blk, ob, it, first_blk, sslot):
                # -- b-pass: prod = hat * out (bcast over i) --
                eng.tensor_mul(prod[:], hat_blk, _ap(ob, [[0, BIb], [1, ND]]))
                # in-place d-tree: contiguous halving over nd slabs
                for half in (256, 128, 64, 32):
                    eng.tensor_add(prod[:, :, 0:half], prod[:, :, 0:half],
                                   prod[:, :, half:2 * half])
                dsum = prod[:, :, 0:N]
                if it == 0:
                    eng.tensor_copy(bbs, dsum)
                else:
                    eng.tensor_add(bbs, bbs, dsum)
                # -- softmax over n (local per (b,i)) --
                ee = worksm.tile([B, BIb, N], BF16, tag=ee_tag)
                rr = worksm.tile([B, BIb], F32, tag=rr_tag)
                nc.scalar.activation(out=ee[:], in_=bbs, func=ACT_EXP,
                                     bias=eps_t, scale=1.0)
                nc.vector.reduce_sum(out=rr[:], in_=ee[:], axis=AX)
                nc.vector.reciprocal(out=rr[:], in_=rr[:])
                cee = worksm.tile([B, BIb, N], BF16, tag=cee_tag)
                eng.tensor_mul(cee[:], ee[:], _ap(rr[:], [[1, BIb], [0, N]]))
                # -- s-pass: prod = hat * c (bcast over d) --
                eng.tensor_mul(prod[:], hat_blk,
                               _ap(cee[:], [[N, BIb], [0, D], [1, N]]))
                # in-place i-tree
                half = BIb // 2
                while half >= 1:
                    eng.tensor_add(prod[:, 0:half, :], prod[:, 0:half, :],
                                   prod[:, half:2 * half, :])
                    half //= 2
                isum = prod[:, 0, :]
                ss = sacc[:, sslot, :]
                if first_blk:
                    eng.tensor_copy(ss, isum)
                else:
                    eng.tensor_add(ss, ss, isum)

            for it in range(2):
                ob = squash(it)
                for blk in range(DVE_NBLK):
                    i0 = blk * DVE_BI
                    route_block(nc.vector, dprod, bb_dve[:, i0:i0 + DVE_BI, :],
                                "dee", "drr", "dcee", DVE_BI,
                                hat[:, i0:i0 + DVE_BI, :], ob, it, blk == 0, 0)
                for blk in range(GP_NBLK):
                    i0 = GP_I0 + blk * GP_BI
                    j0 = i0 - GP_I0
                    route_block(nc.gpsimd, gprod, bb_gp[:, j0:j0 + GP_BI, :],
                                "gee", "grr", "gcee", GP_BI,
                                hat[:, i0:i0 + GP_BI, :], ob, it, blk == 0, 1)
                # s = sacc_dve + sacc_gp (bf16) -> AR payload
                if GP_NBLK:
                    nc.vector.tensor_add(sbf[:], sacc[:, 0, :], sacc[:, 1, :])
                else:
                    nc.vector.tensor_copy(sbf[:], sacc[:, 0, :])
                allreduce_s()

            squash(2)
            nc.sync.dma_start(out=out_d[:], in_=outT[:])

    nc.compile()
    return nc


_NC_CACHE = None


def kernel(inputs: np.ndarray, W: np.ndarray) -> np.ndarray:
    global _NC_CACHE
    if _NC_CACHE is None:
        _NC_CACHE = build_nc()
    nc = _NC_CACHE

    inputs = np.ascontiguousarray(inputs, dtype=np.float32)
    W = np.ascontiguousarray(W, dtype=np.float32)
    bf = ml_dtypes.bfloat16
    in_maps = []
    for c in range(NCORES):
        sl = slice(c * IL, (c + 1) * IL)
        xs = inputs[:, sl, :]                     # [B, IL, 8]
        ws = W[:, sl, :, :]                       # [N, IL, D, 8]
        x_c = np.ascontiguousarray(xs.transpose(2, 1, 0).astype(bf))
        w_c = np.ascontiguousarray(
            ws.transpose(3, 1, 2, 0).astype(bf)).reshape(DIN, IL, ND)
        x2_c = np.ascontiguousarray(
            xs.transpose(1, 2, 0).astype(bf)).reshape(IL * DIN, B)
        w2_c = np.ascontiguousarray(
            ws.transpose(1, 3, 2, 0).astype(bf)).reshape(IL * DIN, ND)
        in_maps.append({"x": x_c, "w": w_c, "x2": x2_c, "w2": w2_c})

    trace = bool(int(os.environ.get("CAPS_TRACE", "0")))
    res = bass_utils.run_bass_kernel_spmd(
        nc, in_maps, core_ids=list(range(NCORES)), trace=trace)
    if trace and res.exec_time_ns is not None:
        print(f"HW exec time: {res.exec_time_ns} ns")
    return res.results[0]["out"].reshape(B, N, D).astype(np.float32)


# revision 9
# speedup vs baseline: 2.4858x; 1.0023x over previous
"""CapsuleLayer dynamic-routing kernel for 8 Trainium2 NeuronCores.

I-sharding: each core owns 144 of the 1152 input capsules.

v2 design (vs v1 baseline at 1.41 ms):
  - All matmuls bf16 (fp32 LOW_HIGH mode was 4x slower per row).
  - hat stored [b, i, d, n] (d-major, n innermost) in bf16. Every routing
    multiply then has an innermost-contiguous 16-bit access pattern ->
    DVE 2x mode; and both reductions (over d and over i) become pure
    contiguous-halving in-place add trees (2x) instead of TensorReduce (1x).
  - s0 = (1/N) sum_i hat computed by a dedicated K=128 GEMM over (i,k)
    so its AllReduce overlaps the whole hat-creation phase.
  - Routing block-parallel across engines: DVE owns 7 blocks of 16 i,
    GpSimd owns 4 blocks of 8 i; ACT does exp + a share of PSUM drains.
  - bb logits and s accumulators in bf16 (gate is 2e-2).
Cross-core: AllReduce of s [128,512] bf16, 3x (first hidden under create).
Every core computes the identical final output; core 0's is returned.
"""

import os
import numpy as np
import ml_dtypes

import concourse.bass as bass
import concourse.bacc as bacc
import concourse.tile as tile
import concourse.mybir as mybir
from concourse import bass_utils

B, I, DIN = 128, 1152, 8
N, D = 32, 16
ND = N * D  # 512, flattened (d, n): nd = d*32 + n
NCORES = 8
IL = I // NCORES  # 144
EPS = 1e-7
F32 = mybir.dt.float32
BF16 = mybir.dt.bfloat16
CH = 6            # i-chunk for X/W streaming in create

# routing block split: DVE gets 7 blocks of 16 i, GpSimd 4 blocks of 8 i
DVE_BI, DVE_NBLK = 16, 9     # all i on DVE
GP_BI, GP_NBLK = 8, 0        # GP disabled (SBUF contention test)
GP_I0 = DVE_BI * DVE_NBLK


def _ap(ap: bass.AP, dims) -> bass.AP:
    """Rebuild `ap` with explicit free [step,count] dims (partition dim kept)."""
    return bass.AP(tensor=ap.tensor, offset=ap.offset, ap=[ap.ap[0]] + list(dims))


def build_nc():
    nc = bacc.Bacc(
        "TRN2",
        target_bir_lowering=False,
        debug=False,
        enable_asserts=True,
        num_devices=NCORES,
    )
    x_d = nc.dram_tensor("x", [DIN, IL, B], BF16, kind="ExternalInput").ap()
    w_d = nc.dram_tensor("w", [DIN, IL, ND], BF16, kind="ExternalInput").ap()
    x2_d = nc.dram_tensor("x2", [IL * DIN, B], BF16, kind="ExternalInput").ap()
    w2_d = nc.dram_tensor("w2", [IL * DIN, ND], BF16, kind="ExternalInput").ap()
    out_d = nc.dram_tensor("out", [B, ND], F32, kind="ExternalOutput").ap()

    AX = mybir.AxisListType.X
    ACT_COPY = mybir.ActivationFunctionType.Copy
    ACT_EXP = mybir.ActivationFunctionType.Exp
    ACT_SQRT = mybir.ActivationFunctionType.Sqrt

    with tile.TileContext(nc) as tc:
        with (
            tc.tile_pool(name="big", bufs=1) as big,
            tc.tile_pool(name="stream", bufs=2) as stream,
            tc.tile_pool(name="workbig", bufs=1) as workbig,
            tc.tile_pool(name="worksm", bufs=2) as worksm,
            tc.tile_pool(name="ps", bufs=5, space="PSUM") as pspool,
            tc.tile_pool(name="ps0", bufs=1, space="PSUM") as ps0pool,
            tc.tile_pool(name="dram", bufs=1, space="DRAM") as dram,
        ):
            hat = big.tile([B, IL, ND], BF16)           # 147.5 KB/part
            bb_dve = big.tile([B, GP_I0, N], BF16)      # 7 KB
            bb_gp = (big.tile([B, IL - GP_I0, N], BF16)
                     if GP_NBLK else None)              # 2 KB
            tsq = big.tile([B, ND], F32)                # 2 KB
            outT = big.tile([B, ND], F32)               # 2 KB
            outbf_t = big.tile([B, 2, ND], BF16)        # 2 KB
            sacc = big.tile([B, 2, ND], BF16)           # 2 KB per-engine s acc
            sbf = big.tile([B, ND], BF16)               # 1 KB AR payload
            s_ar = big.tile([B, ND], BF16)              # 1 KB AR result
            smalls = big.tile([B, 5, N], F32)           # 0.6 KB
            s2, a1, r1, rt = (smalls[:, j, :] for j in range(4))
            eps_t = smalls[:, 4, 0:1]
            # persistent per-engine work tiles (single-buffered: same-engine
            # program order already serializes reuse)
            dprod = workbig.tile([B, DVE_BI, ND], BF16)  # 16 KB
            gprod = (workbig.tile([B, GP_BI, ND], BF16)
                     if GP_NBLK else None)               # 8 KB

            nc.vector.memset(eps_t, EPS)

            # ---------- s0 GEMM:  s0[b, nd] = sum_(i,k) x2 * w2  ----------
            s0ps = ps0pool.tile([B, ND], F32)
            NK = IL * DIN // B  # 9 chunks of K=128
            for j in range(NK):
                x2c = stream.tile([B, B], BF16, tag="x2")
                w2c = stream.tile([B, ND], BF16, tag="w2")
                nc.gpsimd.dma_start(out=x2c[:], in_=x2_d[j * B:(j + 1) * B, :])
                nc.sync.dma_start(out=w2c[:], in_=w2_d[j * B:(j + 1) * B, :])
                nc.tensor.matmul(
                    s0ps[:], lhsT=x2c[:], rhs=w2c[:],
                    start=(j == 0), stop=(j == NK - 1),
                )
            # s0 * (1/N) -> bf16 AR payload
            nc.scalar.activation(out=sbf[:], in_=s0ps[:], func=ACT_COPY,
                                 scale=1.0 / N)

            def allreduce_s():
                ar_in = dram.tile([B, ND], BF16, tag="arin")
                ar_out = dram.tile([B, ND], BF16, tag="arout")
                nc.gpsimd.dma_start(out=ar_in[:], in_=sbf[:])
                nc.gpsimd.collective_compute(
                    "AllReduce",
                    mybir.AluOpType.add,
                    replica_groups=[list(range(NCORES))],
                    ins=[ar_in.opt()],
                    outs=[ar_out.opt()],
                )
                nc.gpsimd.dma_start(out=s_ar[:], in_=ar_out[:])

            allreduce_s()  # AR1 (s0) overlaps the create loop below

            # ---------- create hat ----------
            for ic in range(IL // CH):
                wt = stream.tile([DIN, CH, ND], BF16, tag="wch")
                xch = stream.tile([DIN, CH, B], BF16, tag="xch")
                nc.sync.dma_start(out=wt[:], in_=w_d[:, ic * CH:(ic + 1) * CH, :])
                nc.gpsimd.dma_start(out=xch[:], in_=x_d[:, ic * CH:(ic + 1) * CH, :])
                for j in range(CH):
                    i = ic * CH + j
                    ps = pspool.tile([B, ND], F32)
                    nc.tensor.matmul(
                        ps[:], lhsT=xch[:, j, :], rhs=wt[:, j, :],
                        start=True, stop=True,
                    )
                    if i % 5 < 2:
                        nc.vector.tensor_copy(hat[:, i, :], ps[:])
                    else:
                        nc.scalar.copy(out=hat[:, i, :], in_=ps[:])

            # ---------- squash: out from s (bf16) ----------
            def squash(r):
                last = (r == 2)
                s_in = s_ar[:]
                nc.vector.tensor_mul(tsq[:], s_in, s_in)     # f32 = bf16^2
                # s2[b,n] = sum_d tsq   (d: stride 32, count 16, innermost)
                nc.vector.reduce_sum(
                    out=s2, in_=_ap(tsq[:], [[1, N], [N, D]]), axis=AX)
                nc.scalar.add(out=a1, in_=s2, add=1.0)
                nc.vector.reciprocal(out=r1, in_=a1)
                nc.vector.tensor_mul(r1, r1, s2)              # s2/(1+s2)
                nc.scalar.activation(out=rt, in_=s2, func=ACT_SQRT,
                                     bias=eps_t, scale=1.0)
                nc.vector.reciprocal(out=rt, in_=rt)
                nc.vector.tensor_mul(r1, r1, rt)              # scale [B,N]
                if last:
                    # outT[b, n, d] = s[b, d, n] * r1[b, n]
                    nc.vector.tensor_mul(
                        _ap(outT[:], [[D, N], [1, D]]),
                        _ap(s_in, [[1, N], [N, D]]),
                        _ap(r1, [[1, N], [0, D]]))
                    return None
                ob = outbf_t[:, r % 2, :]
                nc.vector.tensor_mul(
                    ob, s_in, _ap(r1, [[0, D], [1, N]]))      # bcast over d
                return ob

            # ---------- routing ----------
            def route_block(eng, prod, bbs, ee_tag, rr_tag, cee_tag,
                            BIb, hat_blk, ob, it, first_blk, sslot):
                # -- b-pass: prod = hat * out (bcast over i) --
                eng.tensor_mul(prod[:], hat_blk, _ap(ob, [[0, BIb], [1, ND]]))
                # in-place d-tree: contiguous halving over nd slabs
                for half in (256, 128, 64):
                    eng.tensor_add(prod[:, :, 0:half], prod[:, :, 0:half],
                                   prod[:, :, half:2 * half])
                if it == 0:
                    # final level writes bb directly
                    eng.tensor_add(bbs, prod[:, :, 0:N], prod[:, :, N:2 * N])
                else:
                    eng.tensor_add(prod[:, :, 0:N], prod[:, :, 0:N],
                                   prod[:, :, N:2 * N])
                    eng.tensor_add(bbs, bbs, prod[:, :, 0:N])
                # -- softmax over n (local per (b,i)) --
                ee = worksm.tile([B, BIb, N], BF16, tag=ee_tag)
                rr = worksm.tile([B, BIb], F32, tag=rr_tag)
                nc.scalar.activation(out=ee[:], in_=bbs, func=ACT_EXP,
                                     bias=eps_t, scale=1.0)
                nc.vector.reduce_sum(out=rr[:], in_=ee[:], axis=AX)
                nc.vector.reciprocal(out=rr[:], in_=rr[:])
                cee = worksm.tile([B, BIb, N], BF16, tag=cee_tag)
                eng.tensor_mul(cee[:], ee[:], _ap(rr[:], [[1, BIb], [0, N]]))
                # -- s-pass: prod = hat * c (bcast over d) --
                eng.tensor_mul(prod[:], hat_blk,
                               _ap(cee[:], [[N, BIb], [0, D], [1, N]]))
                # in-place i-tree
                half = BIb // 2
                while half >= 2:
                    eng.tensor_add(prod[:, 0:half, :], prod[:, 0:half, :],
                                   prod[:, half:2 * half, :])
                    half //= 2
                ss = sacc[:, sslot, :]
                if first_blk:
                    eng.tensor_add(ss, prod[:, 0, :], prod[:, 1, :])
                else:
                    eng.tensor_add(prod[:, 0, :], prod[:, 0, :], prod[:, 1, :])
                    eng.tensor_add(ss, ss, prod[:, 0, :])

            for it in range(2):
                ob = squash(it)
                for blk in range(DVE_NBLK):
                    i0 = blk * DVE_BI
                    route_block(nc.vector, dprod, bb_dve[:, i0:i0 + DVE_BI, :],
                                "dee", "drr", "dcee", DVE_BI,
                                hat[:, i0:i0 + DVE_BI, :], ob, it, blk == 0, 0)
                for blk in range(GP_NBLK):
                    i0 = GP_I0 + blk * GP_BI
                    j0 = i0 - GP_I0
                    route_block(nc.gpsimd, gprod, bb_gp[:, j0:j0 + GP_BI, :],
                                "gee", "grr", "gcee", GP_BI,
                                hat[:, i0:i0 + GP_BI, :], ob, it, blk == 0, 1)
                # s = sacc_dve + sacc_gp (bf16) -> AR payload
                if GP_NBLK:
                    nc.vector.tensor_add(sbf[:], sacc[:, 0, :], sacc[:, 1, :])
                else:
                    nc.vector.tensor_copy(sbf[:], sacc[:, 0, :])
                allreduce_s()

            squash(2)
            nc.sync.dma_start(out=out_d[:], in_=outT[:])

    nc.compile()
    return nc


_NC_CACHE = None


def kernel(inputs: np.ndarray, W: np.ndarray) -> np.ndarray:
    global _NC_CACHE
    if _NC_CACHE is None:
        _NC_CACHE = build_nc()
    nc = _NC_CACHE

    inputs = np.ascontiguousarray(inputs, dtype=np.float32)
    W = np.ascontiguousarray(W, dtype=np.float32)
    bf = ml_dtypes.bfloat16
    in_maps = []
    for c in range(NCORES):
        sl = slice(c * IL, (c + 1) * IL)
        xs = inputs[:, sl, :]                     # [B, IL, 8]
        ws = W[:, sl, :, :]                       # [N, IL, D, 8]
        x_c = np.ascontiguousarray(xs.transpose(2, 1, 0).astype(bf))
        w_c = np.ascontiguousarray(
            ws.transpose(3, 1, 2, 0).astype(bf)).reshape(DIN, IL, ND)
        x2_c = np.ascontiguousarray(
            xs.transpose(1, 2, 0).astype(bf)).reshape(IL * DIN, B)
        w2_c = np.ascontiguousarray(
            ws.transpose(1, 3, 2, 0).astype(bf)).reshape(IL * DIN, ND)
        in_maps.append({"x": x_c, "w": w_c, "x2": x2_c, "w2": w2_c})

    trace = bool(int(os.environ.get("CAPS_TRACE", "0")))
    res = bass_utils.run_bass_kernel_spmd(
        nc, in_maps, core_ids=list(range(NCORES)), trace=trace)
    if trace and res.exec_time_ns is not None:
        print(f"HW exec time: {res.exec_time_ns} ns")
    return res.results[0]["out"].reshape(B, N, D).astype(np.float32)
